# revision 1
# baseline (speedup 1.0000x reference)
"""MultiHeadSelfAttention Trainium2 Bass kernel, 8-core SPMD.

Reference:
  q,k,v = einsum('bnd,hkd->bhnk', x, W_{q,k,v});  s = q k^T / sqrt(dk)
  p = softmax(s); out = (p v).transpose -> [B,N,H*DK]; out @ Wo^T + bo

Sharding: head-pair per core (core c owns heads 2c, 2c+1, all batches).
Each core computes a partial output projection over its 128 d-columns of
Wo; host sums the 8 partials and adds the bias.

Numerics: matmuls run as float32r (fp22 operand reads, fp32 PSUM accum).
Softmax row-max comes from an f32r scores pass in [q,m] orientation
reduced per chunk on DVE (tensor_reduce from PSUM) or via an ACT bf16
staging copy + DVE 4x-mode accum (engine balance knob); the -max is
folded into the f32r S^T pass as a 65th contraction row, so exp needs no
per-q bias. Denominators come from ones columns appended to V and are
applied via DVE multiply with a gpsimd partition_broadcast of the
reciprocal row.

Scheduling: engine instruction streams execute in emission order, so all
per-batch work is emitted via generators that are round-robin zipped:
the S^T/exp/PV main stream of batch b interleaves step-by-step with the
h1 score sweep of batch b, the out-projection of batch b-1, and the full
prep (projections, v-transpose, h0 sweep) of batch b+1.
"""
import sys

sys.path.insert(0, "/opt/trn_rl_repo")

import numpy as np

import concourse.bass as bass
import concourse.mybir as mybir
import concourse.tile as tile
from concourse import bacc
from concourse.bass_utils import run_bass_kernel_spmd
from concourse.masks import make_identity

B, N, D = 4, 2048, 1024
H, DK = 16, 64
NCORES = 8
HPC = H // NCORES          # heads per core = 2
DP = HPC * DK              # d-slice per core = 128
SCALE = 1.0 / float(np.sqrt(DK))

F32 = mybir.dt.float32
F32R = mybir.dt.float32r
BF16 = mybir.dt.bfloat16

NQT = N // 128             # 16 q tiles per head
NMC = N // 128             # 16 m chunks per head
NHALF = N // 1024          # 2 halves (1024-wide)

import os
PREP_PRIO_OFFSET = int(os.environ.get('PPO', '60'))
V_EVAC_ACT = True          # v evac engine
OUTPROJ_EVAC_ACT = True    # out-projection evacuation engine


def staged(b, h, qt, ms):
    """Sweep chunks routed ACT-stage + DVE-4x instead of DVE-direct.

    ACT copies that are ready during a batch's S^T phase get greedily
    slotted between exp instructions and stretch the critical ACT chain,
    so only chunks that execute in the inter-batch window (the tail of
    the h0 prep sweep, which ring-order runs last) may use ACT. Batch 0's
    h0 sweep runs at startup when ACT is idle: stage all of it.
    """
    stg = os.environ.get('STG', 'more')
    if stg == 'h0only':
        return h == 0 and (ms == 3 or ms == 2)
    if stg == 'more':
        return ms == 3 or (ms == 2 and qt % 4 == 0)
    return ms == 3


def r(ap):
    return ap.bitcast(F32R)


def build_program():
    nc = bacc.Bacc("TRN2", target_bir_lowering=False, debug=False,
                   enable_asserts=False, num_devices=NCORES)

    xT_d = nc.dram_tensor("xT", [B, D, N], F32, kind="ExternalInput")
    wq_d = nc.dram_tensor("wq", [D, DP], F32, kind="ExternalInput")
    wk_d = nc.dram_tensor("wk", [D, DP], F32, kind="ExternalInput")
    wv_d = nc.dram_tensor("wv", [D, DP], F32, kind="ExternalInput")
    wo_d = nc.dram_tensor("wo", [DP, D], F32, kind="ExternalInput")
    ones_d = nc.dram_tensor("ones", [128, N], F32, kind="ExternalInput")
    part_d = nc.dram_tensor("partial", [B, N, D], F32, kind="ExternalOutput")

    with tile.TileContext(nc) as tc:
        build_tile_kernel(nc, tc, xT_d, wq_d, wk_d, wv_d, wo_d, ones_d, part_d)
    nc.compile()
    return nc


def build_tile_kernel(nc, tc, xT_d, wq_d, wk_d, wv_d, wo_d, ones_d, part_d):
    from contextlib import ExitStack
    ctx = ExitStack()

    def cp(use_act, out, in_):
        if use_act:
            nc.scalar.copy(out, in_)
        else:
            nc.vector.tensor_copy(out, in_)

    with ctx:
        # ---- persistent tiles ----
        wpool = ctx.enter_context(tc.tile_pool(name="w", bufs=1))
        # weights stored chunk-major along free dim: [128, 8*128]
        w_sb = {}
        for name, dram in (("wq", wq_d), ("wk", wk_d), ("wv", wv_d)):
            t = wpool.tile([128, D // 128 * DP], F32R, tag=name)
            nc.sync.dma_start(
                out=t[:].rearrange("p (c m) -> p c m", m=DP),
                in_=dram.ap().rearrange("(c p) m -> p c m", p=128).bitcast(F32R),
            )
            w_sb[name] = t
        wo_sb = wpool.tile([DP, D], F32R, tag="wo")
        nc.sync.dma_start(out=wo_sb[:], in_=wo_d.ap()[:].bitcast(F32R))
        id_sb = wpool.tile([128, 128], F32, tag="ident")
        make_identity(nc, id_sb[:])

        # ---- pools ----
        # PSUM: ps_main 3x2 banks (oa + st + st | outproj op) and ps_prep
        # 2x1 banks (projection slices, v-transposes, sweep chunks).
        xt_pool = ctx.enter_context(tc.tile_pool(name="xt", bufs=9))
        ps_main = ctx.enter_context(tc.tile_pool(name="psm", bufs=3, space="PSUM"))
        ps_prep = ctx.enter_context(tc.tile_pool(name="pssq", bufs=2, space="PSUM"))
        augp = ctx.enter_context(tc.tile_pool(name="aug", bufs=8))
        vsbp = ctx.enter_context(tc.tile_pool(name="vsb", bufs=1))
        vaugp = ctx.enter_context(tc.tile_pool(name="vaug", bufs=3))
        pp = ctx.enter_context(tc.tile_pool(name="psb", bufs=4))
        attp = ctx.enter_context(tc.tile_pool(name="att", bufs=2))
        tmpp = ctx.enter_context(tc.tile_pool(name="tmp", bufs=2))
        scrp = ctx.enter_context(tc.tile_pool(name="scr", bufs=4))
        nmp = ctx.enter_context(tc.tile_pool(name="nm", bufs=4))
        outp = ctx.enter_context(tc.tile_pool(name="out", bufs=3))

        state = {}

        def gen_prep(b):
            """Projections + v transpose + h0 sweep for batch b."""
            s = state[b] = {}
            q_aug = [augp.tile([65, N], F32R, tag="aug", name=f"qaug{b}_{h}")
                     for h in range(HPC)]
            k_aug = [augp.tile([65, N], F32R, tag="aug", name=f"kaug{b}_{h}")
                     for h in range(HPC)]
            v_sb = vsbp.tile([128, N], F32, tag="vsb")
            s["q_aug"], s["k_aug"] = q_aug, k_aug

            def evac_qk(aug0, aug1, psq, sl):
                cp(False, aug0[0:64, sl], psq[0:64, :])
                cp(os.environ.get('AUG1', 'dve') == 'act', aug1[0:64, sl], psq[64:128, :])

            for half in range(NHALF):
                xts = []
                for ch in range(8):
                    xt = xt_pool.tile([128, 1024], F32R, tag="xt")
                    nc.sync.dma_start(
                        out=xt[:],
                        in_=xT_d.ap()[b, ch * 128:(ch + 1) * 128,
                                      half * 1024:(half + 1) * 1024
                                      ].bitcast(F32R),
                    )
                    xts.append(xt)
                # k first so the h0 sweep can start as early as possible
                for tname in ("wk", "wq", "wv"):
                    for ns in range(2):
                        psq = ps_prep.tile([128, 512], F32, tag="sq",
                                           name="prj")
                        for ch in range(8):
                            nc.tensor.matmul(
                                psq[:],
                                r(w_sb[tname][:, ch * DP:(ch + 1) * DP]),
                                r(xts[ch][:, ns * 512:(ns + 1) * 512]),
                                start=(ch == 0), stop=(ch == 7),
                            )
                        sl = slice(half * 1024 + ns * 512,
                                   half * 1024 + (ns + 1) * 512)
                        if tname == "wk":
                            evac_qk(k_aug[0], k_aug[1], psq, sl)
                        elif tname == "wq":
                            evac_qk(q_aug[0], q_aug[1], psq, sl)
                        else:
                            cp(os.environ.get('VEV', 'act') == 'act', v_sb[:, sl], psq[:])
                        yield

            for h in range(HPC):
                nc.sync.dma_start(out=k_aug[h][64:65, :],
                                  in_=ones_d.ap()[0:1, :].bitcast(F32R))

            # v transpose -> v_aug chunks [v^T(64) | ones(64)]
            v_aug = [vaugp.tile([128, NMC * 128], F32R, tag="vaug",
                                name=f"vaug{b}_{h}") for h in range(HPC)]
            s["v_aug"] = v_aug
            for h in range(HPC):
                hs = slice(h * DK, (h + 1) * DK)
                nc.sync.dma_start(
                    out=v_aug[h][:].rearrange(
                        "p (c w) -> p c w", w=128)[:, :, DK:],
                    in_=ones_d.ap()[:, 0:NMC * DK].rearrange(
                        "p (c w) -> p c w", w=DK).bitcast(F32R))
                for g in range(4):
                    vt_ps = ps_prep.tile([128, 512], F32, tag="sq",
                                         name="vt_ps")
                    for j in range(4):
                        mc = g * 4 + j
                        nc.tensor.transpose(
                            vt_ps[:, j * 128:j * 128 + DK],
                            v_sb[hs, mc * 128:(mc + 1) * 128],
                            id_sb[hs, hs])
                    cp(False,
                       v_aug[h][:].rearrange(
                           "p (c w) -> p c w",
                           w=128)[:, g * 4:(g + 1) * 4, 0:DK],
                       vt_ps[:].rearrange("p (c w) -> p c w",
                                          w=128)[:, :, 0:DK])
                    yield

            yield from gen_sweep(b, 0)

        def gen_sweep(b, h):
            """f32r scores for head h in [q,m] orientation; per-q row max
            -> negated -> q_aug row 64."""
            q_aug_h = state[b]["q_aug"][h]
            k_aug_h = state[b]["k_aug"][h]
            negmax = nmp.tile([128, 32], F32, tag="nm", name=f"ngm{b}_{h}")
            nc.vector.memset(negmax[:, NQT:], 0.0)
            for qt in range(NQT):
                parts = nmp.tile([128, 4], F32, tag="nmparts")
                d4 = nmp.tile([128, 4], F32, tag="nmd4")
                if h == 0:
                    swp = ps_main.tile([128, 1024], F32, tag="ps2b",
                                       name="swp")
                    for ms in range(2):
                        nc.tensor.matmul(
                            swp[:, ms * 512:(ms + 1) * 512],
                            q_aug_h[0:64, qt * 128:(qt + 1) * 128],
                            k_aug_h[0:64, ms * 512:(ms + 1) * 512],
                            start=True, stop=True,
                        )
                    nc.vector.tensor_reduce(
                        parts[:, 0:1], swp[:],
                        mybir.AxisListType.X, mybir.AluOpType.max)
                ms_lo = 2 if h == 0 else 0
                npart = 3 if h == 0 else 4
                for ms in range(ms_lo, 4):
                    sp = ps_prep.tile([128, 512], F32, tag="sq", name="sq")
                    nc.tensor.matmul(
                        sp[:],
                        q_aug_h[0:64, qt * 128:(qt + 1) * 128],
                        k_aug_h[0:64, ms * 512:(ms + 1) * 512],
                        start=True, stop=True,
                    )
                    if staged(b, h, qt, ms):
                        # ACT stages bf16 to SBUF so the DVE reduce runs
                        # in 4x bf16-SBUF mode instead of 1x fp32-PSUM
                        scr = scrp.tile([128, 512], BF16, tag="scr")
                        nc.scalar.activation(
                            scr[:], sp[:],
                            mybir.ActivationFunctionType.Copy,
                            bias=0.0, scale=1.0)
                        dmy = scrp.tile([128, 512], BF16, tag="scr",
                                        name="dmy")
                        nc.vector.tensor_scalar(
                            dmy[:], scr[:], 0.0, None,
                            mybir.AluOpType.add, mybir.AluOpType.max,
                            accum_out=parts[:, ms - ms_lo // 2:
                                            ms - ms_lo // 2 + 1],
                        )
                    else:
                        nc.vector.tensor_reduce(
                            parts[:, ms - ms_lo // 2:
                                  ms - ms_lo // 2 + 1], sp[:],
                            mybir.AxisListType.X, mybir.AluOpType.max)
                # -max over the chunk maxes, straight into negmax column
                nc.vector.tensor_scalar(
                    d4[:, 0:npart], parts[:, 0:npart], -1.0, None,
                    mybir.AluOpType.mult, mybir.AluOpType.min,
                    accum_out=negmax[:, qt:qt + 1],
                )
                yield
            # [128,16] -max columns -> row via DVE 32x32 transposes + DMA
            nm_t = nmp.tile([32, 128], F32, tag="nmt", name=f"nmt{b}_{h}")
            for i in range(4):
                nc.vector.transpose(nm_t[0:32, 32 * i:32 * i + 32],
                                    negmax[32 * i:32 * i + 32, 0:32])
            nc.gpsimd.dma_start(out=q_aug_h[64:65, :],
                                in_=nm_t[0:NQT, :].bitcast(F32R))

        def gen_unit(b, h, qh):
            s = state[b]
            q_aug_h, k_aug_h = s["q_aug"][h], s["k_aug"][h]
            v_aug_h = s["v_aug"][h]
            att = s["att"]
            qsl = slice(qh * 1024, (qh + 1) * 1024)
            oa = ps_main.tile([128, 1024], F32, tag="ps2b", name="oa")
            for mc in range(NMC):
                st = ps_main.tile([128, 1024], F32, tag="ps2b", name="st")
                for qs in range(2):
                    nc.tensor.matmul(
                        st[:, qs * 512:(qs + 1) * 512],
                        r(k_aug_h[:, mc * 128:(mc + 1) * 128]),
                        r(q_aug_h[:, qh * 1024 + qs * 512:
                                  qh * 1024 + (qs + 1) * 512]),
                        start=True, stop=True,
                    )
                p_sb = pp.tile([128, 1024], F32R, tag="psb")
                nc.scalar.activation(
                    p_sb[:], st[:],
                    mybir.ActivationFunctionType.Exp,
                    bias=0.0, scale=SCALE)
                for qs in range(2):
                    nc.tensor.matmul(
                        oa[:, qs * 512:(qs + 1) * 512],
                        r(v_aug_h[:, mc * 128:(mc + 1) * 128]),
                        r(p_sb[:, qs * 512:(qs + 1) * 512]),
                        start=(mc == 0), stop=(mc == NMC - 1),
                    )
                yield
            # normalize: att rows = oa[0:64] * (1/denom); denom replicated
            # on oa[64:128] via the ones columns of v_aug.
            # oa rows 64:127 all hold the denominator (64 ones columns
            # in v_aug), so a 64-partition reciprocal IS the broadcast.
            recip = tmpp.tile([64, 1024], F32R, tag="recip")
            with nc.allow_low_precision(reason="f32r is 4-byte"):
                nc.vector.reciprocal(recip[:], oa[64:128, :])
            nc.vector.tensor_tensor(
                att[h * 64:(h + 1) * 64, qsl],
                oa[0:64, :], recip[:], mybir.AluOpType.mult)
            yield

        def gen_main(b):
            s = state[b]
            s["att"] = attp.tile([128, N], F32R, tag="att", name=f"att{b}")
            for h in range(HPC):
                for qh in range(NHALF):
                    yield from gen_unit(b, h, qh)

        def gen_outproj(b):
            att = state[b]["att"]
            for nt in range(N // 128):
                op = ps_main.tile([128, 1024], F32, tag="ps2b", name="op")
                for es in range(2):
                    nc.tensor.matmul(
                        op[:, es * 512:(es + 1) * 512],
                        r(att[:, nt * 128:(nt + 1) * 128]),
                        r(wo_sb[:, es * 512:(es + 1) * 512]),
                        start=True, stop=True,
                    )
                ostg = outp.tile([128, 1024], F32, tag="ostg")
                use_act = OUTPROJ_EVAC_ACT and os.environ.get('OPE', '') != 'dve'
                if b == B - 1 and os.environ.get('TAILSPLIT', '0') == '1':
                    use_act = nt % 2 == 0
                cp(use_act, ostg[:], op[:])
                nc.sync.dma_start(
                    out=part_d.ap()[b, nt * 128:(nt + 1) * 128, :],
                    in_=ostg[:],
                )
                yield

        def rr(*gens):
            live = [iter(g) for g in gens]
            while live:
                for g in list(live):
                    try:
                        next(g)
                    except StopIteration:
                        live.remove(g)

        def empty():
            return iter(())

        def alternate(a, b):
            """Zip two generators 1:1 into one, draining the longer."""
            a, b = iter(a), iter(b)
            live = [a, b]
            while live:
                for g in (a, b):
                    if g in live:
                        try:
                            next(g)
                            yield
                        except StopIteration:
                            live.remove(g)

        # ---- driver ----
        # Two independent knobs per instruction: ring-slot order (emission
        # order of tile allocations) and scheduler priority (bass_priority,
        # shiftable via high_priority). The ps_prep ring (2 banks)
        # serializes its tiles in allocation order, so the h1 sweep of
        # batch b and the whole prep of batch b+1 are ring-interleaved
        # and emitted at batch start -- but pushed to a LATE priority band
        # so they only backfill engine holes and never preempt the units'
        # critical ACT exp chain. outproj goes to an even later band.
        for b in range(B):
            with tc.high_priority(offset=PREP_PRIO_OFFSET if b > 0 else 0):
                rr(gen_prep(b))
            rr(gen_sweep(b, 1))
            if b > 0:
                rr(gen_outproj(b - 1))
            rr(gen_main(b))
        rr(gen_outproj(B - 1))


_PROGRAM = None


def _get_program():
    global _PROGRAM
    if _PROGRAM is None:
        _PROGRAM = build_program()
    return _PROGRAM


_ONES = np.ones((128, N), np.float32)


def make_in_maps(x, W_q, W_k, W_v, Wo_w):
    xT = np.ascontiguousarray(np.transpose(
        np.asarray(x, np.float32), (0, 2, 1)))
    in_maps = []
    for c in range(NCORES):
        hs = slice(HPC * c, HPC * (c + 1))
        wq = np.ascontiguousarray(
            np.asarray(W_q[hs], np.float32).reshape(DP, D).T)
        wk = np.ascontiguousarray(
            np.asarray(W_k[hs], np.float32).reshape(DP, D).T)
        wv = np.ascontiguousarray(
            np.asarray(W_v[hs], np.float32).reshape(DP, D).T)
        wo = np.ascontiguousarray(
            np.asarray(Wo_w, np.float32)[:, DP * c:DP * (c + 1)].T)
        in_maps.append({"xT": xT, "wq": wq, "wk": wk, "wv": wv, "wo": wo,
                        "ones": _ONES})
    return in_maps


def kernel(x, W_q, W_k, W_v, Wo_w, Wo_b):
    nc = _get_program()
    in_maps = make_in_maps(x, W_q, W_k, W_v, Wo_w)
    res = run_bass_kernel_spmd(nc, in_maps, list(range(NCORES)))
    out = res.results[0]["partial"].astype(np.float32)
    for c in range(1, NCORES):
        out += res.results[c]["partial"]
    out += np.asarray(Wo_b, np.float32)
    return out



# revision 10
# speedup vs baseline: 1.0180x; 1.0180x over previous
"""MultiHeadSelfAttention Trainium2 Bass kernel, 8-core SPMD.

Reference:
  q,k,v = einsum('bnd,hkd->bhnk', x, W_{q,k,v});  s = q k^T / sqrt(dk)
  p = softmax(s); out = (p v).transpose -> [B,N,H*DK]; out @ Wo^T + bo

Sharding: head-pair per core (core c owns heads 2c, 2c+1, all batches).
Each core computes a partial output projection over its 128 d-columns of
Wo; host sums the 8 partials and adds the bias.

Numerics: matmuls run as float32r (fp22 operand reads, fp32 PSUM accum).
Softmax row-max comes from an f32r scores pass in [q,m] orientation
reduced per chunk on DVE (tensor_reduce from PSUM) or via an ACT bf16
staging copy + DVE 4x-mode accum (engine balance knob); the -max is
folded into the f32r S^T pass as a 65th contraction row, so exp needs no
per-q bias. Denominators come from ones columns appended to V and are
applied via DVE multiply with a gpsimd partition_broadcast of the
reciprocal row.

Scheduling: engine instruction streams execute in emission order, so all
per-batch work is emitted via generators that are round-robin zipped:
the S^T/exp/PV main stream of batch b interleaves step-by-step with the
h1 score sweep of batch b, the out-projection of batch b-1, and the full
prep (projections, v-transpose, h0 sweep) of batch b+1.
"""
import sys

sys.path.insert(0, "/opt/trn_rl_repo")

import numpy as np

import concourse.bass as bass
import concourse.mybir as mybir
import concourse.tile as tile
from concourse import bacc
from concourse.bass_utils import run_bass_kernel_spmd
from concourse.masks import make_identity

B, N, D = 4, 2048, 1024
H, DK = 16, 64
NCORES = 8
HPC = H // NCORES          # heads per core = 2
DP = HPC * DK              # d-slice per core = 128
SCALE = 1.0 / float(np.sqrt(DK))

F32 = mybir.dt.float32
F32R = mybir.dt.float32r
BF16 = mybir.dt.bfloat16

NQT = N // 128             # 16 q tiles per head
NMC = N // 128             # 16 m chunks per head
NHALF = N // 1024          # 2 halves (1024-wide)

import os
PREP_PRIO_OFFSET = int(os.environ.get('PPO', '60'))
V_EVAC_ACT = True          # v evac engine
OUTPROJ_EVAC_ACT = True    # out-projection evacuation engine


def staged(b, h, qt, ms):
    """Sweep chunks routed ACT-stage + DVE-4x instead of DVE-direct.

    ACT copies that are ready during a batch's S^T phase get greedily
    slotted between exp instructions and stretch the critical ACT chain,
    so only chunks that execute in the inter-batch window (the tail of
    the h0 prep sweep, which ring-order runs last) may use ACT. Batch 0's
    h0 sweep runs at startup when ACT is idle: stage all of it.
    """
    stg = os.environ.get('STG', 'more')
    if stg == 'h0only':
        return h == 0 and (ms == 3 or ms == 2)
    if stg == 'more':
        return ms == 3 or (ms == 2 and qt % 4 == 0)
    return ms == 3


def r(ap):
    return ap.bitcast(F32R)


def build_program():
    nc = bacc.Bacc("TRN2", target_bir_lowering=False, debug=False,
                   enable_asserts=False, num_devices=NCORES)

    xT_d = nc.dram_tensor("xT", [B, D, N], F32, kind="ExternalInput")
    wq_d = nc.dram_tensor("wq", [D, DP], F32, kind="ExternalInput")
    wk_d = nc.dram_tensor("wk", [D, DP], F32, kind="ExternalInput")
    wv_d = nc.dram_tensor("wv", [D, DP], F32, kind="ExternalInput")
    wo_d = nc.dram_tensor("wo", [DP, D], F32, kind="ExternalInput")
    ones_d = nc.dram_tensor("ones", [128, N], F32, kind="ExternalInput")
    ones16_d = nc.dram_tensor("ones16", [128, NMC * DK], BF16,
                              kind="ExternalInput")
    part_d = nc.dram_tensor("partial", [B, N, D], BF16, kind="ExternalOutput")

    with tile.TileContext(nc) as tc:
        build_tile_kernel(nc, tc, xT_d, wq_d, wk_d, wv_d, wo_d, ones_d,
                          ones16_d, part_d)
    nc.compile()
    return nc


def build_tile_kernel(nc, tc, xT_d, wq_d, wk_d, wv_d, wo_d, ones_d,
                      ones16_d, part_d):
    from contextlib import ExitStack
    ctx = ExitStack()

    def cp(use_act, out, in_):
        if use_act:
            nc.scalar.copy(out, in_)
        else:
            nc.vector.tensor_copy(out, in_)

    with ctx:
        # ---- persistent tiles ----
        wpool = ctx.enter_context(tc.tile_pool(name="w", bufs=1))
        # weights stored chunk-major along free dim: [128, 8*128]
        w_sb = {}
        for name, dram in (("wq", wq_d), ("wk", wk_d), ("wv", wv_d)):
            t = wpool.tile([128, D // 128 * DP], F32R, tag=name)
            nc.sync.dma_start(
                out=t[:].rearrange("p (c m) -> p c m", m=DP),
                in_=dram.ap().rearrange("(c p) m -> p c m", p=128).bitcast(F32R),
            )
            w_sb[name] = t
        wo_sb = wpool.tile([DP, D], F32R, tag="wo")
        nc.sync.dma_start(out=wo_sb[:], in_=wo_d.ap()[:].bitcast(F32R))
        id_sb = wpool.tile([128, 128], BF16, tag="ident")
        make_identity(nc, id_sb[:])

        # ---- pools ----
        # PSUM: ps_main 3x2 banks (oa + st + st | outproj op) and ps_prep
        # 2x1 banks (projection slices, v-transposes, sweep chunks).
        xt_pool = ctx.enter_context(tc.tile_pool(name="xt", bufs=9))
        ps_main = ctx.enter_context(tc.tile_pool(name="psm", bufs=3, space="PSUM"))
        ps_prep = ctx.enter_context(tc.tile_pool(name="pssq", bufs=2, space="PSUM"))
        augp = ctx.enter_context(tc.tile_pool(name="aug", bufs=8))
        vsbp = ctx.enter_context(tc.tile_pool(name="vsb", bufs=1))
        vaugp = ctx.enter_context(tc.tile_pool(name="vaug", bufs=3))
        pp = ctx.enter_context(tc.tile_pool(name="psb", bufs=4))
        attp = ctx.enter_context(tc.tile_pool(name="att", bufs=2))
        tmpp = ctx.enter_context(tc.tile_pool(name="tmp", bufs=2))
        scrp = ctx.enter_context(tc.tile_pool(name="scr", bufs=4))
        nmp = ctx.enter_context(tc.tile_pool(name="nm", bufs=4))
        outp = ctx.enter_context(tc.tile_pool(name="out", bufs=3))

        state = {}

        def gen_prep(b):
            """Projections + v transpose + h0 sweep for batch b."""
            s = state[b] = {}
            q_aug = [augp.tile([65, N], F32R, tag="aug", name=f"qaug{b}_{h}")
                     for h in range(HPC)]
            k_aug = [augp.tile([65, N], F32R, tag="aug", name=f"kaug{b}_{h}")
                     for h in range(HPC)]
            v_sb = vsbp.tile([128, N], BF16, tag="vsb")
            s["q_aug"], s["k_aug"] = q_aug, k_aug

            def evac_qk(aug0, aug1, psq, sl):
                cp(False, aug0[0:64, sl], psq[0:64, :])
                cp(os.environ.get('AUG1', 'dve') == 'act', aug1[0:64, sl], psq[64:128, :])

            for half in range(NHALF):
                xts = []
                for ch in range(8):
                    xt = xt_pool.tile([128, 1024], F32R, tag="xt")
                    nc.sync.dma_start(
                        out=xt[:],
                        in_=xT_d.ap()[b, ch * 128:(ch + 1) * 128,
                                      half * 1024:(half + 1) * 1024
                                      ].bitcast(F32R),
                    )
                    xts.append(xt)
                # k first so the h0 sweep can start as early as possible
                for tname in ("wk", "wq", "wv"):
                    for ns in range(2):
                        psq = ps_prep.tile([128, 512], F32, tag="sq",
                                           name="prj")
                        for ch in range(8):
                            nc.tensor.matmul(
                                psq[:],
                                r(w_sb[tname][:, ch * DP:(ch + 1) * DP]),
                                r(xts[ch][:, ns * 512:(ns + 1) * 512]),
                                start=(ch == 0), stop=(ch == 7),
                            )
                        sl = slice(half * 1024 + ns * 512,
                                   half * 1024 + (ns + 1) * 512)
                        if tname == "wk":
                            evac_qk(k_aug[0], k_aug[1], psq, sl)
                        elif tname == "wq":
                            evac_qk(q_aug[0], q_aug[1], psq, sl)
                        else:
                            cp(os.environ.get('VEV', 'act') == 'act', v_sb[:, sl], psq[:])
                        yield

            for h in range(HPC):
                nc.sync.dma_start(out=k_aug[h][64:65, :],
                                  in_=ones_d.ap()[0:1, :].bitcast(F32R))

            # v transpose -> v_aug chunks [v^T(64) | ones(64)]
            v_aug = [vaugp.tile([128, NMC * 128], BF16, tag="vaug",
                                name=f"vaug{b}_{h}") for h in range(HPC)]
            s["v_aug"] = v_aug
            for h in range(HPC):
                hs = slice(h * DK, (h + 1) * DK)
                nc.sync.dma_start(
                    out=v_aug[h][:].rearrange(
                        "p (c w) -> p c w", w=128)[:, :, DK:],
                    in_=ones16_d.ap()[:, 0:NMC * DK].rearrange(
                        "p (c w) -> p c w", w=DK))
                for g in range(4):
                    vt_ps = ps_prep.tile([128, 512], BF16, tag="sq",
                                         name="vt_ps")
                    for j in range(4):
                        mc = g * 4 + j
                        nc.tensor.transpose(
                            vt_ps[:, j * 128:j * 128 + DK],
                            v_sb[hs, mc * 128:(mc + 1) * 128],
                            id_sb[hs, hs])
                    cp(False,
                       v_aug[h][:].rearrange(
                           "p (c w) -> p c w",
                           w=128)[:, g * 4:(g + 1) * 4, 0:DK],
                       vt_ps[:].rearrange("p (c w) -> p c w",
                                          w=128)[:, :, 0:DK])
                    yield

            yield from gen_sweep(b, 0)

        def gen_sweep(b, h):
            """f32r scores for head h in [q,m] orientation; per-q row max
            -> negated -> q_aug row 64."""
            q_aug_h = state[b]["q_aug"][h]
            k_aug_h = state[b]["k_aug"][h]
            negmax = nmp.tile([128, 32], F32, tag="nm", name=f"ngm{b}_{h}")
            nc.vector.memset(negmax[:, NQT:], 0.0)
            for qt in range(NQT):
                parts = nmp.tile([128, 4], F32, tag="nmparts")
                d4 = nmp.tile([128, 4], F32, tag="nmd4")
                if h == 0:
                    swp = ps_main.tile([128, 1024], F32, tag="ps2b",
                                       name="swp")
                    for ms in range(2):
                        nc.tensor.matmul(
                            swp[:, ms * 512:(ms + 1) * 512],
                            q_aug_h[0:64, qt * 128:(qt + 1) * 128],
                            k_aug_h[0:64, ms * 512:(ms + 1) * 512],
                            start=True, stop=True,
                        )
                    nc.vector.tensor_reduce(
                        parts[:, 0:1], swp[:],
                        mybir.AxisListType.X, mybir.AluOpType.max)
                ms_lo = 2 if h == 0 else 0
                npart = 3 if h == 0 else 4
                for ms in range(ms_lo, 4):
                    sp = ps_prep.tile([128, 512], F32, tag="sq", name="sq")
                    nc.tensor.matmul(
                        sp[:],
                        q_aug_h[0:64, qt * 128:(qt + 1) * 128],
                        k_aug_h[0:64, ms * 512:(ms + 1) * 512],
                        start=True, stop=True,
                    )
                    if staged(b, h, qt, ms):
                        # ACT stages bf16 to SBUF so the DVE reduce runs
                        # in 4x bf16-SBUF mode instead of 1x fp32-PSUM
                        scr = scrp.tile([128, 512], BF16, tag="scr")
                        nc.scalar.activation(
                            scr[:], sp[:],
                            mybir.ActivationFunctionType.Copy,
                            bias=0.0, scale=1.0)
                        dmy = scrp.tile([128, 512], BF16, tag="scr",
                                        name="dmy")
                        nc.vector.tensor_scalar(
                            dmy[:], scr[:], 0.0, None,
                            mybir.AluOpType.add, mybir.AluOpType.max,
                            accum_out=parts[:, ms - ms_lo // 2:
                                            ms - ms_lo // 2 + 1],
                        )
                    else:
                        nc.vector.tensor_reduce(
                            parts[:, ms - ms_lo // 2:
                                  ms - ms_lo // 2 + 1], sp[:],
                            mybir.AxisListType.X, mybir.AluOpType.max)
                # -max over the chunk maxes, straight into negmax column
                nc.vector.tensor_scalar(
                    d4[:, 0:npart], parts[:, 0:npart], -1.0, None,
                    mybir.AluOpType.mult, mybir.AluOpType.min,
                    accum_out=negmax[:, qt:qt + 1],
                )
                yield
            # [128,16] -max columns -> row via DVE 32x32 transposes + DMA
            nm_t = nmp.tile([32, 128], F32, tag="nmt", name=f"nmt{b}_{h}")
            for i in range(4):
                nc.vector.transpose(nm_t[0:32, 32 * i:32 * i + 32],
                                    negmax[32 * i:32 * i + 32, 0:32])
            nc.gpsimd.dma_start(out=q_aug_h[64:65, :],
                                in_=nm_t[0:NQT, :].bitcast(F32R))

        def gen_unit(b, h, qh):
            s = state[b]
            q_aug_h, k_aug_h = s["q_aug"][h], s["k_aug"][h]
            v_aug_h = s["v_aug"][h]
            att = s["att"]
            qsl = slice(qh * 1024, (qh + 1) * 1024)
            oa = ps_main.tile([128, 1024], F32, tag="ps2b", name="oa")
            for mc in range(NMC):
                st = ps_main.tile([128, 1024], F32, tag="ps2b", name="st")
                for qs in range(2):
                    nc.tensor.matmul(
                        st[:, qs * 512:(qs + 1) * 512],
                        r(k_aug_h[:, mc * 128:(mc + 1) * 128]),
                        r(q_aug_h[:, qh * 1024 + qs * 512:
                                  qh * 1024 + (qs + 1) * 512]),
                        start=True, stop=True,
                    )
                p_sb = pp.tile([128, 1024], BF16, tag="psb")
                nc.scalar.activation(
                    p_sb[:], st[:],
                    mybir.ActivationFunctionType.Exp,
                    bias=0.0, scale=SCALE)
                for qs in range(2):
                    nc.tensor.matmul(
                        oa[:, qs * 512:(qs + 1) * 512],
                        v_aug_h[:, mc * 128:(mc + 1) * 128],
                        p_sb[:, qs * 512:(qs + 1) * 512],
                        start=(mc == 0), stop=(mc == NMC - 1),
                    )
                yield
            # normalize: att rows = oa[0:64] * (1/denom); denom replicated
            # on oa[64:128] via the ones columns of v_aug.
            # oa rows 64:127 all hold the denominator (64 ones columns
            # in v_aug), so a 64-partition reciprocal IS the broadcast.
            recip = tmpp.tile([64, 1024], F32R, tag="recip")
            with nc.allow_low_precision(reason="f32r is 4-byte"):
                nc.vector.reciprocal(recip[:], oa[64:128, :])
            nc.vector.tensor_tensor(
                att[h * 64:(h + 1) * 64, qsl],
                oa[0:64, :], recip[:], mybir.AluOpType.mult)
            yield

        def gen_main(b):
            s = state[b]
            s["att"] = attp.tile([128, N], F32R, tag="att", name=f"att{b}")
            for h in range(HPC):
                for qh in range(NHALF):
                    yield from gen_unit(b, h, qh)

        def gen_outproj(b):
            att = state[b]["att"]
            for nt in range(N // 128):
                op = ps_main.tile([128, 1024], F32, tag="ps2b", name="op")
                for es in range(2):
                    nc.tensor.matmul(
                        op[:, es * 512:(es + 1) * 512],
                        r(att[:, nt * 128:(nt + 1) * 128]),
                        r(wo_sb[:, es * 512:(es + 1) * 512]),
                        start=True, stop=True,
                    )
                ostg = outp.tile([128, 1024], BF16, tag="ostg")
                use_act = OUTPROJ_EVAC_ACT and os.environ.get('OPE', '') != 'dve'
                if b == B - 1 and os.environ.get('TAILSPLIT', '0') == '1':
                    use_act = nt % 2 == 0
                cp(use_act, ostg[:], op[:])
                nc.sync.dma_start(
                    out=part_d.ap()[b, nt * 128:(nt + 1) * 128, :],
                    in_=ostg[:],
                )
                yield

        def rr(*gens):
            live = [iter(g) for g in gens]
            while live:
                for g in list(live):
                    try:
                        next(g)
                    except StopIteration:
                        live.remove(g)

        def empty():
            return iter(())

        def alternate(a, b):
            """Zip two generators 1:1 into one, draining the longer."""
            a, b = iter(a), iter(b)
            live = [a, b]
            while live:
                for g in (a, b):
                    if g in live:
                        try:
                            next(g)
                            yield
                        except StopIteration:
                            live.remove(g)

        # ---- driver ----
        # Two independent knobs per instruction: ring-slot order (emission
        # order of tile allocations) and scheduler priority (bass_priority,
        # shiftable via high_priority). The ps_prep ring (2 banks)
        # serializes its tiles in allocation order, so the h1 sweep of
        # batch b and the whole prep of batch b+1 are ring-interleaved
        # and emitted at batch start -- but pushed to a LATE priority band
        # so they only backfill engine holes and never preempt the units'
        # critical ACT exp chain. outproj goes to an even later band.
        drv = os.environ.get('DRV', 'seq')
        for b in range(B):
            with tc.high_priority(offset=PREP_PRIO_OFFSET if b > 0 else 0):
                rr(gen_prep(b))
            rr(gen_sweep(b, 1))
            if drv == 'alt':
                if b > 0:
                    rr(alternate(gen_main(b), gen_outproj(b - 1)))
                else:
                    rr(gen_main(b))
            else:
                if b > 0:
                    rr(gen_outproj(b - 1))
                rr(gen_main(b))
        rr(gen_outproj(B - 1))


_PROGRAM = None


def _get_program():
    global _PROGRAM
    if _PROGRAM is None:
        _PROGRAM = build_program()
    return _PROGRAM


_ONES = np.ones((128, N), np.float32)
try:
    import ml_dtypes
    _ONES16 = np.ones((128, NMC * DK), ml_dtypes.bfloat16)
except ImportError:
    _ONES16 = None


def make_in_maps(x, W_q, W_k, W_v, Wo_w):
    xT = np.ascontiguousarray(np.transpose(
        np.asarray(x, np.float32), (0, 2, 1)))
    in_maps = []
    for c in range(NCORES):
        hs = slice(HPC * c, HPC * (c + 1))
        wq = np.ascontiguousarray(
            np.asarray(W_q[hs], np.float32).reshape(DP, D).T)
        wk = np.ascontiguousarray(
            np.asarray(W_k[hs], np.float32).reshape(DP, D).T)
        wv = np.ascontiguousarray(
            np.asarray(W_v[hs], np.float32).reshape(DP, D).T)
        wo = np.ascontiguousarray(
            np.asarray(Wo_w, np.float32)[:, DP * c:DP * (c + 1)].T)
        in_maps.append({"xT": xT, "wq": wq, "wk": wk, "wv": wv, "wo": wo,
                        "ones": _ONES, "ones16": _ONES16})
    return in_maps


def kernel(x, W_q, W_k, W_v, Wo_w, Wo_b):
    nc = _get_program()
    in_maps = make_in_maps(x, W_q, W_k, W_v, Wo_w)
    res = run_bass_kernel_spmd(nc, in_maps, list(range(NCORES)))
    out = np.asarray(res.results[0]["partial"], np.float32)
    for c in range(1, NCORES):
        out += np.asarray(res.results[c]["partial"], np.float32)
    out += np.asarray(Wo_b, np.float32)
    return out

